# revision 33
# baseline (speedup 1.0000x reference)
"""DynamicSincConv1d Trainium2 kernel.

Data-parallel over batch: 8 batch elements -> 8 NeuronCores, one full
pipeline per core. All heavy math (conv, sinc synthesis, DFTs, complex
filtering, overlap-add) runs on-device in 16-bit (fp16/bf16) matmuls
(1 cyc/col on the PE vs fp32's 4); the host only packs inputs into
DMA-friendly fp16 layouts and reassembles the output.

Structure:
 - Stage1 conv packs the 4 hop-shifts into the 128-wide PE output; a PE
   permutation-fold (identity-slice stationaries accumulating 4 shifted
   reads) recombines them; leaky-relu via vector ops.
 - Stage2 tanh emits interleaved rows: mv64[2q]=wid, mv64[2q+1]=1 (via a
   +20 tanh bias on odd columns of the widened stationary), and
   amp duplicated in pairs so rai -> mv2i = (wid*ra, ra) pairs with one
   vector multiply.
 - Sinc stage: u and u2 are built by ONE constant 2-row stationary each
   (sua/sel2a) against the 2-partition pair mv64[2q:2q+2] / mv2i, so the
   selector matrices need no DMA and the PE never reloads weights
   between broadcasts; scalar computes sn = Sin((ta[d]/64)*u); a custom
   fused DVE op (DIV1NR_ANT) computes ft = sn * approx(1/u2).
 - The four sinc planes are pre-summed (2 gpsimd + 1 vector add) before
   the DFT, which therefore needs only ONE accumulation per (mt, chunk):
   R = M1 @ sum_s ft_s.
 - The filter spectra are real ((-1)^fb * R[fb]); the complex multiply
   is a real elementwise product done on the DVE straight out of PSUM.
 - iSTFT overlap-add is folded into the matmul accumulation; the hann^2
   COLA sum is 1.5 except at two edge columns (folded + efix). Each
   output channel's iSTFT + bias + DMA-out is interleaved into the main
   loop as soon as its two cmuls are complete.
 - The f=1024 tail frame is batched across all 32 planes as [128,32]
   ops with the same constant pair stationaries.
"""

import math
import numpy as np

B, CI, I, O, S = 8, 32, 2, 4, 4
K, HOP, T = 256, 64, 65536
F = T // HOP + 1            # 1025 frames
H = (T + K) // HOP          # 1028 hops in the padded signal
PI = math.pi
PIEPS = PI * 1e-6
SU = 64.0                   # u pre-scale (keeps eps rows fp16-normal)
S2 = 256.0                  # u2 pre-scale (keeps 1/u2 inside fp16 range)
CLAMP = 60000.0
FCH = [(0, 512), (512, 512)]   # main chunks; f=1024 tail handled separately

_prog_cache = {}

DIV_C0, DIV_C1 = -0.2346, 2.002


def _install_div_op():
    # Register fused out = in1 * approx(1/in0) custom-DVE op (1 Newton pass).
    if "divop" in _prog_cache:
        return _prog_cache["divop"]
    import concourse.dve_ops as dve_ops
    from concourse.dve_spec import AluOp, Bin, Spec, Src0, Src1, lower, _has_src1
    from concourse.dve_uop import DveOpSpec

    for op in dve_ops.OPS:
        if op.name == "DIV1NR_ANT":
            _prog_cache["divop"] = op
            return op

    _nx = Bin(AluOp.BITWISE_NOT, Src0, Src0)
    _y0 = _nx * dve_ops.C0
    _body = (_y0 * (dve_ops.C1 - Src0 * _y0)) * Src1

    def _ref_div1(in0, in1, c0, c1, c2):
        nx = (~in0.view(np.int32)).view(np.float32)
        y0 = nx * c0
        return (y0 * (c1 - in0 * y0)) * in1

    spec = Spec(body=_body, reference=_ref_div1)
    row = max(dve_ops._SUB_OPCODE_FOR_NAME.values()) + 1
    shas = {}
    for ver in ("v3", "v4"):
        u = lower(spec, ver=ver)
        shas[ver] = DveOpSpec(name="DIV1NR_ANT", opcode=row, uops=u,
                              rd1_en=_has_src1(spec)).sha(ver)
    op = dve_ops.DveOp("DIV1NR_ANT", spec, subdim=False, uops_sha=shas)
    dve_ops.OPS.append(op)
    dve_ops.CUSTOM_DVE_SPECS[op.name] = op.spec
    dve_ops._SUB_OPCODE_FOR_NAME[op.name] = row
    _prog_cache["divop"] = op
    return op


def _ta():
    ta = PI * np.arange(128, dtype=np.float64) / K
    ta[0] = 1.0
    return ta


def _consts():
    n = np.arange(K, dtype=np.float64)
    ola = 0.5 * (1.0 - np.cos(2.0 * np.pi * n / K))
    fir = 0.42 - 0.5 * np.cos(2.0 * np.pi * n / K) + 0.08 * np.cos(4.0 * np.pi * n / K)

    d = np.arange(128, dtype=np.float64)
    fb = np.arange(K // 2 + 1, dtype=np.float64)     # 0..128
    cd = np.where(d == 0, 1.0, 2.0)
    # M1[d, fb] = (-1)^fb * c_d * fir[128+d]/(S*K) * cos(2*pi*d*fb/K)
    m1 = (((-1.0) ** fb)[None, :] * cd[:, None] * fir[128 + d.astype(int)][:, None]
          / (S * K) * np.cos(2.0 * np.pi * np.outer(d, fb) / K))
    ta = _ta()
    m1f = m1 * (S2 / ta)[:, None]                     # [128, 129]
    m1fa = np.ascontiguousarray(m1f[:, 0:128]).astype(np.float16)
    m1fb = np.concatenate([m1f[:, 128:129], m1f[:, 1:128]], axis=1).astype(np.float16)

    # STFT stationaries: wxp[r, (mt*4+j)*128 + fb'] (fbpack col layout)
    kk = np.arange(K, dtype=np.float64)
    ang = 2.0 * np.pi * np.outer(kk, fb) / K          # [256, 129]
    wre = ola[:, None] * np.cos(ang)                  # [256, 129]
    wim = -ola[:, None] * np.sin(ang)
    colsA = wre[:, 0:128]                             # [256, 128]
    colsB = np.concatenate([wre[:, 128:129], wim[:, 1:128]], axis=1)
    wx_full = np.stack([colsA, colsB], axis=0)        # [2 mt, 256 k, 128]
    wxp = np.zeros((64, 1024), dtype=np.float64)
    for mt in range(2):
        for j in range(4):
            wxp[:, (mt * 4 + j) * 128:(mt * 4 + j + 1) * 128] = \
                wx_full[mt, 64 * j:64 * j + 64, :]
    wxp = wxp.astype(np.float16)

    # iSTFT: IC[fbpack_row, n] with ola folded, pre-scaled
    nn_ = np.arange(K, dtype=np.float64)
    cp = np.where(fb == 0, 1.0, 2.0)
    icre = (cp[:, None] / K) * np.cos(2.0 * np.pi * np.outer(fb, nn_) / K) * ola[None, :]
    icim = (-2.0 / K) * np.sin(2.0 * np.pi * np.outer(fb, nn_) / K) * ola[None, :]
    ica = icre[0:128] * (1.0 / 1.5)                                        # [128, 256]
    icb = np.concatenate([icre[128:129], icim[1:128]], axis=0) * (1.0 / 1.5)
    icj = np.zeros((128, 512), dtype=np.float64)
    for ab, icm in ((0, ica), (1, icb)):
        for jj in range(4):
            icj[:, (ab * 4 + jj) * 64:(ab * 4 + jj + 1) * 64] = \
                icm[:, 64 * jj:64 * jj + 64]
    icj = icj.astype(np.float16)

    # env is exactly 1.5 for p in [1,1022] (periodic-hann COLA); fold 1/1.5
    # into icj and fix the p=0 / p=1023 columns with per-partition scales.
    ola2 = ola * ola
    env_q = np.zeros((H, 64), dtype=np.float64)
    for j in range(4):
        env_q[j:F + j, :] += ola2[64 * j:64 * j + 64][None, :]
    envt = env_q[2:2 + 1024, :].T                      # [64 r, 1024 p]
    efix = np.stack([1.5 / envt[:, 0], 1.5 / envt[:, 1023]], axis=1)
    efix = np.ascontiguousarray(efix).astype(np.float32)   # [64, 2]

    # selector stationaries for u and u2 broadcasts (per q = oi*4+s)
    epsova = PIEPS / ta                                # [128]; ta[0]=1 -> PIEPS
    selu = np.zeros((33, 32 * 128), dtype=np.float64)
    sel2 = np.zeros((64, 32 * 128), dtype=np.float64)
    for q in range(32):
        c0 = q * 128
        selu[q, c0 + 1:c0 + 128] = SU                  # col d=0 stays 0
        selu[32, c0:c0 + 128] = SU * epsova
        selu[32, c0] = SU * PIEPS
        sel2[q, c0 + 1:c0 + 128] = S2
        sel2[32 + q, c0:c0 + 128] = S2 * epsova
        sel2[32 + q, c0] = S2 * PIEPS
    selu = selu.astype(np.float16)
    sel2 = sel2.astype(np.float16)

    scale_ta = (_ta() / SU).reshape(128, 1).astype(np.float32)
    scale_ta[0, 0] = 1.0 / SU

    ident = np.eye(128, dtype=np.float16)

    # f=1024 tail via DVE outer-sums: tam = ta*[d>=1], s2m = S2*[d>=1],
    # s2e = S2*epsova, broadcast over the 32 q-partitions
    dmask = (d >= 1).astype(np.float64)
    tailc = np.zeros((32, 384), dtype=np.float64)
    tailc[:, 0:128] = (ta * dmask)[None, :]
    tailc[:, 128:256] = (S2 * dmask)[None, :]
    tailc[:, 256:384] = (S2 * epsova)[None, :]
    tailc = tailc.astype(np.float16)
    sel48 = np.zeros((32, 16), dtype=np.float16)
    for q in range(32):
        sel48[q, q // 4] = 1.0

    return dict(m1fa=m1fa, m1fb=m1fb, wxp=wxp, icj=icj, efix=efix,
                selu=selu, sel2=sel2, scale_ta=scale_ta, ident=ident,
                tailc=tailc, sel48=sel48)


def _build_program():
    import concourse.bacc as bacc
    import concourse.mybir as mybir
    import concourse.tile as tile

    f32 = mybir.dt.float32
    f16 = mybir.dt.float16
    bf16 = mybir.dt.bfloat16
    AF = mybir.ActivationFunctionType

    divop = _install_div_op()
    nc = bacc.Bacc("TRN2", target_bir_lowering=False, debug=False, num_devices=8)

    d_in = nc.dram_tensor("d_in", [16, 128, H], f16, kind="ExternalInput")
    xd_in = nc.dram_tensor("xd_in", [64, 2 * H], f16, kind="ExternalInput")
    w4_in = nc.dram_tensor("w4_in", [128, 2048], f16, kind="ExternalInput")
    w2tw_in = nc.dram_tensor("w2tw_in", [32, 32], f16, kind="ExternalInput")
    w2ta_in = nc.dram_tensor("w2ta_in", [32, 64], f16, kind="ExternalInput")
    ident_in = nc.dram_tensor("ident_in", [128, 128], f16, kind="ExternalInput")
    b1_in = nc.dram_tensor("b1_in", [32, 1], f32, kind="ExternalInput")
    b2a_in = nc.dram_tensor("b2a_in", [64, 1], f32, kind="ExternalInput")
    b2w_in = nc.dram_tensor("b2w_in", [32, 1], f32, kind="ExternalInput")
    ta_in = nc.dram_tensor("ta_in", [128, 1], f32, kind="ExternalInput")
    selu_in = nc.dram_tensor("selu_in", [33, 4096], f16, kind="ExternalInput")
    sel2_in = nc.dram_tensor("sel2_in", [64, 4096], f16, kind="ExternalInput")
    m1fa_in = nc.dram_tensor("m1fa_in", [128, 128], f16, kind="ExternalInput")
    m1fb_in = nc.dram_tensor("m1fb_in", [128, 128], f16, kind="ExternalInput")
    wxp_in = nc.dram_tensor("wxp_in", [64, 1024], f16, kind="ExternalInput")
    icj_in = nc.dram_tensor("icj_in", [128, 512], f16, kind="ExternalInput")
    tailc_in = nc.dram_tensor("tailc_in", [32, 384], f16, kind="ExternalInput")
    sel48_in = nc.dram_tensor("sel48_in", [32, 16], f16, kind="ExternalInput")
    efix_in = nc.dram_tensor("efix_in", [64, 2], f32, kind="ExternalInput")
    bias_in = nc.dram_tensor("bias_in", [64, 4], f32, kind="ExternalInput")
    yt_out = nc.dram_tensor("yt_out", [64, 4096], f32, kind="ExternalOutput")

    with nc.allow_low_precision(reason="output tolerance 2e-2; 16-bit pipeline"), \
         tile.TileContext(nc) as tc:
        with tc.tile_pool(name="cpool", bufs=1) as cpool:
            w4_sb = cpool.tile([128, 2048], f16, tag="w4")
            w2tw_sb = cpool.tile([32, 32], f16, tag="w2tw")
            w2ta_sb = cpool.tile([32, 64], f16, tag="w2ta")
            ident_sb = cpool.tile([128, 128], f16, tag="ident")
            b1_sb = cpool.tile([32, 1], f32, tag="b1")
            b2a_sb = cpool.tile([64, 1], f32, tag="b2a")
            b2w_sb = cpool.tile([32, 1], f32, tag="b2w")
            ta_sb = cpool.tile([128, 1], f32, tag="ta")
            selu_sb = cpool.tile([33, 4096], f16, tag="selu")
            sel2_sb = cpool.tile([64, 4096], f16, tag="sel2")
            m1fa_sb = cpool.tile([128, 128], f16, tag="m1fa")
            m1fb_sb = cpool.tile([128, 128], f16, tag="m1fb")
            wxp_sb = cpool.tile([64, 1024], f16, tag="wxp")
            icj_sb = cpool.tile([128, 512], f16, tag="icj")
            tailc_sb = cpool.tile([32, 384], f16, tag="tailc")
            sel48_sb = cpool.tile([32, 16], f16, tag="sel48")
            efix_sb = cpool.tile([64, 2], f32, tag="efix")
            bias_sb = cpool.tile([64, 4], f32, tag="bias")
            xd_sb = cpool.tile([64, 2 * H], f16, tag="xd")

            wup_sb = cpool.tile([1, 640], f16, tag="wup")
            nc.vector.memset(wup_sb[:], 0.0)
            with tc.tile_pool(name="pswu", bufs=1, space="PSUM") as pswu:
                pw0 = pswu.tile([128, 512], f32, tag="pw0")
                for wi in range(16):
                    nc.tensor.matmul(pw0[:], wup_sb[:, 0:128], wup_sb[:, 128:640],
                                     start=True, stop=True)

            for t_sb, t_in in ((w4_sb, w4_in), (ident_sb, ident_in),
                               (xd_sb, xd_in), (wxp_sb, wxp_in)):
                nc.sync.dma_start(t_sb[:], t_in[:])
            dts = []
            with tc.tile_pool(name="dpool", bufs=1) as dpool_pre:
                for c in range(16):
                    dtile = dpool_pre.tile([128, H], f16, tag=f"d{c}", name=f"d{c}")
                    nc.sync.dma_start(dtile[:], d_in[c])
                    dts.append(dtile)
            for t_sb, t_in in ((w2tw_sb, w2tw_in), (w2ta_sb, w2ta_in),
                               (b1_sb, b1_in), (b2a_sb, b2a_in), (b2w_sb, b2w_in),
                               (ta_sb, ta_in), (selu_sb, selu_in),
                               (sel2_sb, sel2_in),
                               (m1fa_sb, m1fa_in), (m1fb_sb, m1fb_in),
                               (icj_sb, icj_in),
                               (tailc_sb, tailc_in), (sel48_sb, sel48_in),
                               (efix_sb, efix_in), (bias_sb, bias_in)):
                nc.sync.dma_start(t_sb[:], t_in[:])

            # persistent SBUF tiles
            h_sb = cpool.tile([32, F], f16, tag="h")
            sA_sb = cpool.tile([128, H], f16, tag="sA")
            mv_sb = cpool.tile([33, H], f16, tag="mv")        # wid rows + ones
            mv2_sb = cpool.tile([64, H], f16, tag="mv2")      # wioa + ra rows
            amp0_sb = cpool.tile([64, F], f32, tag="amp0")
            ra32_sb = cpool.tile([64, F], f32, tag="ra32")
            ra16_sb = cpool.tile([32, F], f16, tag="ra16")
            xa_sb = cpool.tile([128, 2 * F], f16, tag="xa")
            xb_sb = cpool.tile([128, 2 * F], f16, tag="xb")
            rta_sb = cpool.tile([128, 8], f16, tag="rta")     # R tail (a), col=oi
            rtb_sb = cpool.tile([128, 8], f16, tag="rtb")
            utp_sb = cpool.tile([32, 128], f16, tag="utp")
            tcol_sb = cpool.tile([32, 4], f32, tag="tcol")
            snq_sb = cpool.tile([32, 128], f16, tag="snq")
            u2a_sb = cpool.tile([32, 128], f16, tag="u2a")
            u2b_sb = cpool.tile([32, 128], f16, tag="u2b")
            ftq_sb = cpool.tile([32, 128], bf16, tag="ftq")
            ft8_sb = cpool.tile([16, 128], f16, tag="ft8")
            fttT_sb = cpool.tile([128, 16], f16, tag="fttT")
            yt_sb = cpool.tile([64, 4096], f32, tag="yt")
            ya_t = [cpool.tile([128, H], f16, tag=f"ya{o}", name=f"ya{o}") for o in range(4)]
            yb_t = [cpool.tile([128, H], f16, tag=f"yb{o}", name=f"yb{o}") for o in range(4)]
            hp1 = cpool.tile([32, F], f16, tag="hp1")
            hp2 = cpool.tile([32, F], f16, tag="hp2")

            # ---- stage 1a: conv with j packed in PE output ----
            with tc.tile_pool(name="ps1a", bufs=2, space="PSUM") as ps1a:
                for (f0, nf) in [(0, 512), (512, 512), (1024, 4)]:
                    ps = ps1a.tile([128, 512], f32, tag="ps1a")
                    for cb in range(16):
                        nc.tensor.matmul(
                            ps[:, 0:nf],
                            w4_sb[:, cb * 128:(cb + 1) * 128],
                            dts[cb][:, f0:f0 + nf],
                            start=(cb == 0), stop=(cb == 15))
                    nc.scalar.activation(sA_sb[:, f0:f0 + nf], ps[:, 0:nf],
                                         AF.Copy)

            # ---- stage 1b: fold 4 shifted j groups via PE; Lrelu(x + b1) ----
            # chunk edges chosen so fold chunk k only reads sA chunks <= k
            # (each fold chunk's copy overlaps the next chunk's matmuls)
            with tc.tile_pool(name="psf", bufs=2, space="PSUM") as psf:
                for (f0, nf) in [(0, 509), (509, 512), (1021, 4)]:
                    ph = psf.tile([32, 512], f32, tag="psf")
                    for j in range(4):
                        nc.tensor.matmul(ph[:, 0:nf],
                                         ident_sb[:, 32 * j:32 * (j + 1)],
                                         sA_sb[:, f0 + j:f0 + j + nf],
                                         start=(j == 0), stop=(j == 3))
                    nc.vector.tensor_scalar(hp1[:, f0:f0 + nf], ph[:, 0:nf],
                                            b1_sb[:, 0:1], None,
                                            mybir.AluOpType.add)
            nc.vector.tensor_scalar(hp2[:], hp1[:], 0.01, None,
                                    mybir.AluOpType.mult)
            nc.vector.tensor_max(h_sb[:], hp1[:], hp2[:])

            # zero the pad columns of ya/yb (cmul writes 1..1025 only)
            nc.gpsimd.memset(mv_sb[32:33, :], 1.0)
            for o in range(4):
                for yd in (ya_t[o], yb_t[o]):
                    nc.gpsimd.memset(yd[:, 0:1], 0.0)
                    nc.gpsimd.memset(yd[:, 1026:1028], 0.0)

            # ---- STFT halves around stage2 + tail chain: the PE streams
            # STFT matmuls while scalar/vector run tanh -> recip -> mv2 ----
            def stft_half(ps4, i, cp_i):
                for (mt, xdst) in ((0, xa_sb), (1, xb_sb)):
                    for (f0, nf) in FCH:
                        ps = ps4.tile([128, 512], f32, tag="ps4")
                        for j in range(4):
                            nc.tensor.matmul(
                                ps[:, 0:nf],
                                wxp_sb[:, (mt * 4 + j) * 128:(mt * 4 + j + 1) * 128],
                                xd_sb[:, i * H + f0 + j:i * H + f0 + j + nf],
                                start=(j == 0), stop=(j == 3))
                        if cp_i % 2 == 0:
                            nc.vector.tensor_copy(
                                xdst[:, i * F + f0:i * F + f0 + nf], ps[:, 0:nf])
                        else:
                            nc.scalar.activation(
                                xdst[:, i * F + f0:i * F + f0 + nf],
                                ps[:, 0:nf], AF.Copy)
                        cp_i += 1
                return cp_i

            with tc.tile_pool(name="ps4", bufs=2, space="PSUM") as ps4, \
                 tc.tile_pool(name="ps2", bufs=2, space="PSUM") as ps2, \
                 tc.tile_pool(name="pstl", bufs=1, space="PSUM") as pstl:
                cp_i = stft_half(ps4, 0, 0)

                # stage 2: 1x1 conv + tanh -> wid (mv rows 0-31), amp0
                for (f0, nf) in [(0, 512), (512, 512), (1024, 1)]:
                    pw = ps2.tile([32, 512], f32, tag="ps2w")
                    nc.tensor.matmul(pw[:, 0:nf], w2tw_sb[:], h_sb[:, f0:f0 + nf],
                                     start=True, stop=True)
                    nc.scalar.activation(mv_sb[0:32, f0:f0 + nf], pw[:, 0:nf],
                                         AF.Tanh, bias=b2w_sb[:, 0:1])
                    pa = ps2.tile([64, 512], f32, tag="ps2a")
                    nc.tensor.matmul(pa[:, 0:nf], w2ta_sb[:], h_sb[:, f0:f0 + nf],
                                     start=True, stop=True)
                    nc.scalar.activation(amp0_sb[:, f0:f0 + nf], pa[:, 0:nf],
                                         AF.Tanh, bias=b2a_sb[:, 0:1])

                # mv prep: ra = clamp(1/amp), wioa = wid*ra (overlaps STFT i=1)
                nc.vector.reciprocal_approx_fast(ra32_sb[:], amp0_sb[:])
                nc.vector.tensor_scalar_min(ra32_sb[:], ra32_sb[:], CLAMP)
                nc.vector.tensor_scalar_max(ra16_sb[:], ra32_sb[0:32, :], -CLAMP)
                nc.vector.tensor_scalar_max(mv2_sb[32:64, 0:F],
                                            ra32_sb[32:64, :], -CLAMP)
                nc.vector.tensor_mul(mv2_sb[0:32, 0:F], mv_sb[0:32, 0:F],
                                     ra16_sb[:])

                cp_i = stft_half(ps4, 1, cp_i)
                # STFT tail f=1024 (batched: cols = i*2 + mt)
                pst = ps4.tile([128, 4], f32, tag="ps4")
                for i in range(2):
                    for mt in range(2):
                        for j in range(4):
                            nc.tensor.matmul(
                                pst[:, i * 2 + mt:i * 2 + mt + 1],
                                wxp_sb[:, (mt * 4 + j) * 128:(mt * 4 + j + 1) * 128],
                                xd_sb[:, i * H + 1024 + j:i * H + 1024 + j + 1],
                                start=(j == 0), stop=(j == 3))
                for i in range(2):
                    nc.scalar.activation(xa_sb[:, i * F + 1024:i * F + 1025],
                                         pst[:, i * 2:i * 2 + 1], AF.Copy)
                    nc.scalar.activation(xb_sb[:, i * F + 1024:i * F + 1025],
                                         pst[:, i * 2 + 1:i * 2 + 2], AF.Copy)

                # f=1024 tail on the DVE: u' = ta*wid + PIEPS (per q row),
                # u2 = S2*(wioa + ra*epsova); ft = sin(u')*approx(1/u2);
                # one selector matmul sums s and a transpose-DMA flips to
                # [d, oi] for the tail DFT.  No tiny-matmul storm.
                nc.vector.memset(tcol_sb[:, 3:4], PIEPS)
                nc.vector.tensor_copy(tcol_sb[:, 0:1], mv_sb[0:32, 1024:1025])
                nc.vector.tensor_copy(tcol_sb[:, 1:2], ra16_sb[:, 1024:1025])
                nc.vector.tensor_copy(tcol_sb[:, 2:3], mv2_sb[0:32, 1024:1025])
                nc.vector.tensor_scalar(utp_sb[:], tailc_sb[:, 0:128],
                                        tcol_sb[:, 0:1], None,
                                        mybir.AluOpType.mult)
                nc.scalar.activation(snq_sb[:], utp_sb[:], AF.Sin,
                                     bias=tcol_sb[:, 3:4])
                nc.vector.tensor_scalar(u2a_sb[:], tailc_sb[:, 256:384],
                                        tcol_sb[:, 1:2], None,
                                        mybir.AluOpType.mult)
                nc.vector.scalar_tensor_tensor(
                    u2b_sb[:], tailc_sb[:, 128:256],
                    tcol_sb[:, 2:3], u2a_sb[:],
                    mybir.AluOpType.mult, mybir.AluOpType.add)
                nc.vector._custom_dve(divop, out=ftq_sb[:], in0=u2b_sb[:],
                                      in1=snq_sb[:], s0=DIV_C0, s1=DIV_C1,
                                      imm2=0.0)
                pqt = pstl.tile([16, 128], f32, tag="pqt")
                nc.tensor.matmul(pqt[:], sel48_sb[:], ftq_sb[:],
                                 start=True, stop=True)
                nc.scalar.activation(ft8_sb[:], pqt[:], AF.Copy)
                nc.sync.dma_start_transpose(fttT_sb[:], ft8_sb[:])

            # ---- main loop: u/u2 -> sin/recip -> ft -> presum -> DFT -> cmul ----
            with tc.tile_pool(name="psu", bufs=2, space="PSUM") as psu, \
                 tc.tile_pool(name="psv", bufs=3, space="PSUM") as psv, \
                 tc.tile_pool(name="psr", bufs=2, space="PSUM") as psr, \
                 tc.tile_pool(name="psy", bufs=1, space="PSUM") as psy, \
                 tc.tile_pool(name="snp", bufs=5) as snp, \
                 tc.tile_pool(name="ftp", bufs=10) as ftp, \
                 tc.tile_pool(name="ctp", bufs=3) as ctp:
                def emit_chain(oi):
                    fts = []
                    for s in range(4):
                        q = oi * 4 + s
                        pvs = []
                        for (f0, nf) in FCH:
                            pv = psv.tile([128, 512], f32, tag="pv")
                            nc.tensor.matmul(pv[:, 0:nf],
                                             sel2_sb[:, q * 128:(q + 1) * 128],
                                             mv2_sb[0:64, f0:f0 + nf],
                                             start=True, stop=True)
                            pvs.append(pv)
                        sn = snp.tile([128, 1024], f16, tag="sn")
                        for (f0, nf) in FCH:
                            pu = psu.tile([128, 512], f32, tag="pu")
                            nc.tensor.matmul(pu[:, 0:nf],
                                             selu_sb[:, q * 128:(q + 1) * 128],
                                             mv_sb[0:33, f0:f0 + nf],
                                             start=True, stop=True)
                            nc.scalar.activation(sn[:, f0:f0 + nf], pu[:, 0:nf],
                                                 AF.Sin, scale=ta_sb[:, 0:1])
                        ft = ftp.tile([128, 1024], bf16, tag="ft")
                        for ci, (f0, nf) in enumerate(FCH):
                            nc.vector._custom_dve(divop, out=ft[:, f0:f0 + nf],
                                                  in0=pvs[ci][:, 0:nf],
                                                  in1=sn[:, f0:f0 + nf],
                                                  s0=DIV_C0, s1=DIV_C1, imm2=0.0)
                        fts.append(ft)
                    return fts

                def emit_dft_cmul(oi, fts):
                    o, il = oi // 2, oi % 2
                    for (mt, m1sb, rt, xsb, ydst) in (
                            (0, m1fa_sb, rta_sb, xa_sb, ya_t[o]),
                            (1, m1fb_sb, rtb_sb, xb_sb, yb_t[o])):
                        for (f0, nf) in FCH:
                            ps = psr.tile([128, 512], f32, tag="psr")
                            for s in range(4):
                                nc.tensor.matmul(ps[:, 0:nf], m1sb[:],
                                                 fts[s][:, f0:f0 + nf],
                                                 start=(s == 0), stop=(s == 3))
                            # cmul straight from PSUM (R is real)
                            if il == 0:
                                nc.vector.tensor_mul(
                                    ydst[:, 1 + f0:1 + f0 + nf],
                                    xsb[:, il * F + f0:il * F + f0 + nf],
                                    ps[:, 0:nf])
                            else:
                                tmp = ctp.tile([128, 512], f16, tag="ctmp")
                                nc.vector.tensor_mul(
                                    tmp[:, 0:nf],
                                    xsb[:, il * F + f0:il * F + f0 + nf],
                                    ps[:, 0:nf])
                                nc.gpsimd.tensor_add(ydst[:, 1 + f0:1 + f0 + nf],
                                                     ydst[:, 1 + f0:1 + f0 + nf],
                                                     tmp[:, 0:nf])
                        # f = 1024 tail column from rta/rtb
                        if il == 0:
                            nc.vector.tensor_mul(ydst[:, 1025:1026],
                                                 xsb[:, il * F + 1024:il * F + 1025],
                                                 rt[:, oi:oi + 1])
                        else:
                            tmp = ctp.tile([128, 512], f16, tag="ctmp")
                            nc.vector.tensor_mul(tmp[:, 0:1],
                                                 xsb[:, il * F + 1024:il * F + 1025],
                                                 rt[:, oi:oi + 1])
                            nc.gpsimd.tensor_add(ydst[:, 1025:1026],
                                                 ydst[:, 1025:1026], tmp[:, 0:1])

                def emit_istft(o):
                    for (c0, nf) in FCH:
                        py = psy.tile([64, 512], f32, tag="py")
                        for jj in range(4):
                            for (ab, ysrc) in ((0, ya_t), (1, yb_t)):
                                st = icj_sb[:, (ab * 4 + jj) * 64:(ab * 4 + jj + 1) * 64]
                                nc.tensor.matmul(
                                    py[:, 0:nf], st,
                                    ysrc[o][:, c0 + 3 - jj:c0 + 3 - jj + nf],
                                    start=(jj == 0 and ab == 0),
                                    stop=(jj == 3 and ab == 1))
                        if c0 == 0:
                            nc.vector.tensor_scalar(py[:, 0:1], py[:, 0:1],
                                                    efix_sb[:, 0:1], None,
                                                    mybir.AluOpType.mult)
                        else:
                            nc.vector.tensor_scalar(py[:, 511:512], py[:, 511:512],
                                                    efix_sb[:, 1:2], None,
                                                    mybir.AluOpType.mult)
                        nc.scalar.activation(
                            yt_sb[:, o * 1024 + c0:o * 1024 + c0 + nf],
                            py[:, 0:nf],
                            AF.Identity, bias=bias_sb[:, o:o + 1])
                        nc.sync.dma_start(
                            yt_out[:, o * 1024 + c0:o * 1024 + c0 + nf],
                            yt_sb[:, o * 1024 + c0:o * 1024 + c0 + nf])

                prev_fts = emit_chain(0)
                # tail DFT: rta/rtb from the s-summed ftt4 (PE already busy
                # with chain(0) while scalar/DVE produced ftt4)
                prt = psr.tile([128, 512], f32, tag="psr")
                nc.tensor.matmul(prt[:, 0:8], m1fa_sb[:], fttT_sb[:, 0:8],
                                 start=True, stop=True)
                nc.tensor.matmul(prt[:, 8:16], m1fb_sb[:], fttT_sb[:, 0:8],
                                 start=True, stop=True)
                nc.scalar.activation(rta_sb[:], prt[:, 0:8], AF.Copy)
                nc.scalar.activation(rtb_sb[:], prt[:, 8:16], AF.Copy)

                for oi in range(1, 8):
                    fts = emit_chain(oi)
                    emit_dft_cmul(oi - 1, prev_fts)
                    if (oi - 1) in (2, 4, 6):
                        emit_istft((oi - 1) // 2 - 1)
                    prev_fts = fts
                emit_dft_cmul(7, prev_fts)
                emit_istft(3)

    nc.compile()
    return nc


def _prep_inputs(x, conditioning, w1, b1, w2, b2, bias):
    c = _consts()
    x = np.asarray(x, dtype=np.float32)
    conditioning = np.asarray(conditioning, dtype=np.float32)
    w1 = np.asarray(w1, dtype=np.float32)
    b1 = np.asarray(b1, dtype=np.float32)
    w2 = np.asarray(w2, dtype=np.float32)
    b2 = np.asarray(b2, dtype=np.float32)
    bias = np.asarray(bias, dtype=np.float32)

    # W4[cb, 64*(c-2cb)+r, 32j+o] = w1[o, c, 64j+r]
    arr = w1.reshape(32, 32, 4, 64)                      # [o, c, j, r]
    W4 = arr.transpose(1, 3, 2, 0).reshape(32, 64, 4 * 32)  # [c, r, (j,o)]
    W4 = W4.reshape(16, 128, 128)                        # [cb, row, col]
    w4 = np.ascontiguousarray(
        W4.transpose(1, 0, 2).reshape(128, 2048)).astype(np.float16)

    w2m = w2[:, :, 0]                                    # [64, 32]
    w2tw = np.ascontiguousarray(w2m[32:64].T).astype(np.float16)
    w2ta1 = w2m[0:32].T                                  # [32, 32]
    w2ta = np.ascontiguousarray(
        np.concatenate([w2ta1, w2ta1], axis=1)).astype(np.float16)  # [32, 64]
    bias64 = np.tile(bias.reshape(1, 4), (64, 1)).astype(np.float32)

    shared = {
        "w4_in": w4,
        "w2tw_in": w2tw,
        "w2ta_in": w2ta,
        "ident_in": c["ident"],
        "b1_in": b1.reshape(32, 1).copy(),
        "b2a_in": np.tile(b2[:32], 2).reshape(64, 1).copy(),
        "b2w_in": b2[32:].reshape(32, 1).copy(),
        "ta_in": c["scale_ta"],
        "selu_in": c["selu"], "sel2_in": c["sel2"],
        "m1fa_in": c["m1fa"], "m1fb_in": c["m1fb"],
        "wxp_in": c["wxp"], "icj_in": c["icj"], "efix_in": c["efix"],
        "tailc_in": c["tailc"], "sel48_in": c["sel48"],
        "bias_in": bias64,
    }
    in_maps = []
    for b in range(B):
        condpad = np.zeros((CI, T + K), dtype=np.float32)
        condpad[:, 128:128 + T] = conditioning[b]
        d = condpad.reshape(CI, H, 64).transpose(0, 2, 1).reshape(2048, H)
        d = np.ascontiguousarray(d.reshape(16, 128, H)).astype(np.float16)
        xp = np.pad(x[b], ((0, 0), (128, 128)), mode="reflect")
        xd = np.ascontiguousarray(
            xp.reshape(2, H, 64).transpose(0, 2, 1).reshape(2, 64, H)
            .transpose(1, 0, 2).reshape(64, 2 * H)).astype(np.float16)
        m = dict(shared)
        m["d_in"] = d
        m["xd_in"] = xd
        in_maps.append(m)
    return in_maps


def _assemble(results):
    y = np.empty((B, O, T), dtype=np.float32)
    for b in range(B):
        yt = results[b]["yt_out"]                        # [64, 4096]
        y[b] = yt.reshape(64, 4, 1024).transpose(1, 2, 0).reshape(4, T)
    return y


def kernel(x, conditioning, w1, b1, w2, b2, bias):
    from concourse.bass_utils import run_bass_kernel_spmd
    if "nc" not in _prog_cache:
        _prog_cache["nc"] = _build_program()
    nc = _prog_cache["nc"]
    in_maps = _prep_inputs(x, conditioning, w1, b1, w2, b2, bias)
    res = run_bass_kernel_spmd(nc, in_maps, core_ids=list(range(B)))
    return _assemble(res.results)


# revision 34
# speedup vs baseline: 1.0901x; 1.0901x over previous
"""DynamicSincConv1d Trainium2 kernel.

Data-parallel over batch: 8 batch elements -> 8 NeuronCores, one full
pipeline per core. All heavy math (conv, sinc synthesis, DFTs, complex
filtering, overlap-add) runs on-device in 16-bit (fp16/bf16) matmuls
(1 cyc/col on the PE vs fp32's 4); the host only packs inputs into
DMA-friendly fp16 layouts and reassembles the output.

Structure:
 - Stage1 conv packs the 4 hop-shifts into the 128-wide PE output; a PE
   permutation-fold (identity-slice stationaries accumulating 4 shifted
   reads) recombines them; leaky-relu via vector ops.
 - Stage2 tanh emits interleaved rows: mv64[2q]=wid, mv64[2q+1]=1 (via a
   +20 tanh bias on odd columns of the widened stationary), and
   amp duplicated in pairs so rai -> mv2i = (wid*ra, ra) pairs with one
   vector multiply.
 - Sinc stage: u and u2 are built by ONE constant 2-row stationary each
   (sua/sel2a) against the 2-partition pair mv64[2q:2q+2] / mv2i, so the
   selector matrices need no DMA and the PE never reloads weights
   between broadcasts; scalar computes sn = Sin((ta[d]/64)*u); a custom
   fused DVE op (DIV1NR_ANT) computes ft = sn * approx(1/u2).
 - The four sinc planes are pre-summed (2 gpsimd + 1 vector add) before
   the DFT, which therefore needs only ONE accumulation per (mt, chunk):
   R = M1 @ sum_s ft_s.
 - The filter spectra are real ((-1)^fb * R[fb]); the complex multiply
   is a real elementwise product done on the DVE straight out of PSUM.
 - iSTFT overlap-add is folded into the matmul accumulation; the hann^2
   COLA sum is 1.5 except at two edge columns (folded + efix). Each
   output channel's iSTFT + bias + DMA-out is interleaved into the main
   loop as soon as its two cmuls are complete.
 - The f=1024 tail frame is batched across all 32 planes as [128,32]
   ops with the same constant pair stationaries.
"""

import math
import numpy as np

B, CI, I, O, S = 8, 32, 2, 4, 4
K, HOP, T = 256, 64, 65536
F = T // HOP + 1            # 1025 frames
H = (T + K) // HOP          # 1028 hops in the padded signal
PI = math.pi
PIEPS = PI * 1e-6
SU = 64.0                   # u pre-scale (keeps eps rows fp16-normal)
S2 = 256.0                  # u2 pre-scale (keeps 1/u2 inside fp16 range)
CLAMP = 60000.0
FCH = [(0, 512), (512, 512)]   # main chunks; f=1024 tail handled separately

_prog_cache = {}

DIV_C0, DIV_C1 = -0.2346, 2.002


def _install_div_op():
    # Register fused out = in1 * approx(1/in0) custom-DVE op (1 Newton pass).
    if "divop" in _prog_cache:
        return _prog_cache["divop"]
    import concourse.dve_ops as dve_ops
    from concourse.dve_spec import AluOp, Bin, Spec, Src0, Src1, lower, _has_src1
    from concourse.dve_uop import DveOpSpec

    for op in dve_ops.OPS:
        if op.name == "DIV1NR_ANT":
            _prog_cache["divop"] = op
            return op

    _nx = Bin(AluOp.BITWISE_NOT, Src0, Src0)
    _y0 = _nx * dve_ops.C0
    _body = (_y0 * (dve_ops.C1 - Src0 * _y0)) * Src1

    def _ref_div1(in0, in1, c0, c1, c2):
        nx = (~in0.view(np.int32)).view(np.float32)
        y0 = nx * c0
        return (y0 * (c1 - in0 * y0)) * in1

    spec = Spec(body=_body, reference=_ref_div1)
    row = max(dve_ops._SUB_OPCODE_FOR_NAME.values()) + 1
    shas = {}
    for ver in ("v3", "v4"):
        u = lower(spec, ver=ver)
        shas[ver] = DveOpSpec(name="DIV1NR_ANT", opcode=row, uops=u,
                              rd1_en=_has_src1(spec)).sha(ver)
    op = dve_ops.DveOp("DIV1NR_ANT", spec, subdim=False, uops_sha=shas)
    dve_ops.OPS.append(op)
    dve_ops.CUSTOM_DVE_SPECS[op.name] = op.spec
    dve_ops._SUB_OPCODE_FOR_NAME[op.name] = row
    _prog_cache["divop"] = op
    return op


def _ta():
    ta = PI * np.arange(128, dtype=np.float64) / K
    ta[0] = 1.0
    return ta


def _consts():
    n = np.arange(K, dtype=np.float64)
    ola = 0.5 * (1.0 - np.cos(2.0 * np.pi * n / K))
    fir = 0.42 - 0.5 * np.cos(2.0 * np.pi * n / K) + 0.08 * np.cos(4.0 * np.pi * n / K)

    d = np.arange(128, dtype=np.float64)
    fb = np.arange(K // 2 + 1, dtype=np.float64)     # 0..128
    cd = np.where(d == 0, 1.0, 2.0)
    # M1[d, fb] = (-1)^fb * c_d * fir[128+d]/(S*K) * cos(2*pi*d*fb/K)
    m1 = (((-1.0) ** fb)[None, :] * cd[:, None] * fir[128 + d.astype(int)][:, None]
          / (S * K) * np.cos(2.0 * np.pi * np.outer(d, fb) / K))
    ta = _ta()
    m1f = m1 * (S2 / ta)[:, None]                     # [128, 129]
    m1fa = np.ascontiguousarray(m1f[:, 0:128]).astype(np.float16)
    m1fb = np.concatenate([m1f[:, 128:129], m1f[:, 1:128]], axis=1).astype(np.float16)

    # STFT stationaries: wxp[r, (mt*4+j)*128 + fb'] (fbpack col layout)
    kk = np.arange(K, dtype=np.float64)
    ang = 2.0 * np.pi * np.outer(kk, fb) / K          # [256, 129]
    wre = ola[:, None] * np.cos(ang)                  # [256, 129]
    wim = -ola[:, None] * np.sin(ang)
    colsA = wre[:, 0:128]                             # [256, 128]
    colsB = np.concatenate([wre[:, 128:129], wim[:, 1:128]], axis=1)
    wx_full = np.stack([colsA, colsB], axis=0)        # [2 mt, 256 k, 128]
    wxp = np.zeros((64, 1024), dtype=np.float64)
    for mt in range(2):
        for j in range(4):
            wxp[:, (mt * 4 + j) * 128:(mt * 4 + j + 1) * 128] = \
                wx_full[mt, 64 * j:64 * j + 64, :]
    wxp = wxp.astype(np.float16)

    # iSTFT: IC[fbpack_row, n] with ola folded, pre-scaled
    nn_ = np.arange(K, dtype=np.float64)
    cp = np.where(fb == 0, 1.0, 2.0)
    icre = (cp[:, None] / K) * np.cos(2.0 * np.pi * np.outer(fb, nn_) / K) * ola[None, :]
    icim = (-2.0 / K) * np.sin(2.0 * np.pi * np.outer(fb, nn_) / K) * ola[None, :]
    ica = icre[0:128] * (1.0 / 1.5)                                        # [128, 256]
    icb = np.concatenate([icre[128:129], icim[1:128]], axis=0) * (1.0 / 1.5)
    icj = np.zeros((128, 512), dtype=np.float64)
    for ab, icm in ((0, ica), (1, icb)):
        for jj in range(4):
            icj[:, (ab * 4 + jj) * 64:(ab * 4 + jj + 1) * 64] = \
                icm[:, 64 * jj:64 * jj + 64]
    icj = icj.astype(np.float16)

    # env is exactly 1.5 for p in [1,1022] (periodic-hann COLA); fold 1/1.5
    # into icj and fix the p=0 / p=1023 columns with per-partition scales.
    ola2 = ola * ola
    env_q = np.zeros((H, 64), dtype=np.float64)
    for j in range(4):
        env_q[j:F + j, :] += ola2[64 * j:64 * j + 64][None, :]
    envt = env_q[2:2 + 1024, :].T                      # [64 r, 1024 p]
    efix = np.stack([1.5 / envt[:, 0], 1.5 / envt[:, 1023]], axis=1)
    efix = np.ascontiguousarray(efix).astype(np.float32)   # [64, 2]

    # selector stationaries for u and u2 broadcasts (per q = oi*4+s)
    epsova = PIEPS / ta                                # [128]; ta[0]=1 -> PIEPS
    selu = np.zeros((33, 32 * 128), dtype=np.float64)
    sel2 = np.zeros((64, 32 * 128), dtype=np.float64)
    for q in range(32):
        c0 = q * 128
        selu[q, c0 + 1:c0 + 128] = SU                  # col d=0 stays 0
        selu[32, c0:c0 + 128] = SU * epsova
        selu[32, c0] = SU * PIEPS
        sel2[q, c0 + 1:c0 + 128] = S2
        sel2[32 + q, c0:c0 + 128] = S2 * epsova
        sel2[32 + q, c0] = S2 * PIEPS
    selu = selu.astype(np.float16)
    sel2 = sel2.astype(np.float16)

    scale_ta = (_ta() / SU).reshape(128, 1).astype(np.float32)
    scale_ta[0, 0] = 1.0 / SU

    ident = np.eye(128, dtype=np.float16)

    # f=1024 tail via DVE outer-sums: tam = ta*[d>=1], s2m = S2*[d>=1],
    # s2e = S2*epsova, broadcast over the 32 q-partitions
    dmask = (d >= 1).astype(np.float64)
    tailc = np.zeros((32, 384), dtype=np.float64)
    tailc[:, 0:128] = (ta * dmask)[None, :]
    tailc[:, 128:256] = (S2 * dmask)[None, :]
    tailc[:, 256:384] = (S2 * epsova)[None, :]
    tailc = tailc.astype(np.float16)
    sel48 = np.zeros((32, 16), dtype=np.float16)
    for q in range(32):
        sel48[q, q // 4] = 1.0

    return dict(m1fa=m1fa, m1fb=m1fb, wxp=wxp, icj=icj, efix=efix,
                selu=selu, sel2=sel2, scale_ta=scale_ta, ident=ident,
                tailc=tailc, sel48=sel48)


def _build_program():
    import concourse.bacc as bacc
    import concourse.mybir as mybir
    import concourse.tile as tile

    f32 = mybir.dt.float32
    f16 = mybir.dt.float16
    bf16 = mybir.dt.bfloat16
    AF = mybir.ActivationFunctionType

    divop = _install_div_op()
    nc = bacc.Bacc("TRN2", target_bir_lowering=False, debug=False, num_devices=8)

    d_in = nc.dram_tensor("d_in", [16, 128, H], f16, kind="ExternalInput")
    xd_in = nc.dram_tensor("xd_in", [64, 2 * H], f16, kind="ExternalInput")
    w4_in = nc.dram_tensor("w4_in", [128, 2048], f16, kind="ExternalInput")
    w2tw_in = nc.dram_tensor("w2tw_in", [32, 32], f16, kind="ExternalInput")
    w2ta_in = nc.dram_tensor("w2ta_in", [32, 64], f16, kind="ExternalInput")
    ident_in = nc.dram_tensor("ident_in", [128, 128], f16, kind="ExternalInput")
    b1_in = nc.dram_tensor("b1_in", [32, 1], f32, kind="ExternalInput")
    b2a_in = nc.dram_tensor("b2a_in", [64, 1], f32, kind="ExternalInput")
    b2w_in = nc.dram_tensor("b2w_in", [32, 1], f32, kind="ExternalInput")
    ta_in = nc.dram_tensor("ta_in", [128, 1], f32, kind="ExternalInput")
    selu_in = nc.dram_tensor("selu_in", [33, 4096], f16, kind="ExternalInput")
    sel2_in = nc.dram_tensor("sel2_in", [64, 4096], f16, kind="ExternalInput")
    m1fa_in = nc.dram_tensor("m1fa_in", [128, 128], f16, kind="ExternalInput")
    m1fb_in = nc.dram_tensor("m1fb_in", [128, 128], f16, kind="ExternalInput")
    wxp_in = nc.dram_tensor("wxp_in", [64, 1024], f16, kind="ExternalInput")
    icj_in = nc.dram_tensor("icj_in", [128, 512], f16, kind="ExternalInput")
    tailc_in = nc.dram_tensor("tailc_in", [32, 384], f16, kind="ExternalInput")
    sel48_in = nc.dram_tensor("sel48_in", [32, 16], f16, kind="ExternalInput")
    efix_in = nc.dram_tensor("efix_in", [64, 2], f32, kind="ExternalInput")
    bias_in = nc.dram_tensor("bias_in", [64, 4], f32, kind="ExternalInput")
    yt_out = nc.dram_tensor("yt_out", [64, 4096], f32, kind="ExternalOutput")

    with nc.allow_low_precision(reason="output tolerance 2e-2; 16-bit pipeline"), \
         tile.TileContext(nc) as tc:
        with tc.tile_pool(name="cpool", bufs=1) as cpool:
            w4_sb = cpool.tile([128, 2048], f16, tag="w4")
            w2tw_sb = cpool.tile([32, 32], f16, tag="w2tw")
            w2ta_sb = cpool.tile([32, 64], f16, tag="w2ta")
            ident_sb = cpool.tile([128, 128], f16, tag="ident")
            b1_sb = cpool.tile([32, 1], f32, tag="b1")
            b2a_sb = cpool.tile([64, 1], f32, tag="b2a")
            b2w_sb = cpool.tile([32, 1], f32, tag="b2w")
            ta_sb = cpool.tile([128, 1], f32, tag="ta")
            selu_sb = cpool.tile([33, 4096], f16, tag="selu")
            sel2_sb = cpool.tile([64, 4096], f16, tag="sel2")
            m1fa_sb = cpool.tile([128, 128], f16, tag="m1fa")
            m1fb_sb = cpool.tile([128, 128], f16, tag="m1fb")
            wxp_sb = cpool.tile([64, 1024], f16, tag="wxp")
            icj_sb = cpool.tile([128, 512], f16, tag="icj")
            tailc_sb = cpool.tile([32, 384], f16, tag="tailc")
            sel48_sb = cpool.tile([32, 16], f16, tag="sel48")
            efix_sb = cpool.tile([64, 2], f32, tag="efix")
            bias_sb = cpool.tile([64, 4], f32, tag="bias")
            xd_sb = cpool.tile([64, 2 * H], f16, tag="xd")

            wup_sb = cpool.tile([1, 640], f16, tag="wup")
            nc.vector.memset(wup_sb[:], 0.0)
            with tc.tile_pool(name="pswu", bufs=1, space="PSUM") as pswu:
                pw0 = pswu.tile([128, 512], f32, tag="pw0")
                for wi in range(16):
                    nc.tensor.matmul(pw0[:], wup_sb[:, 0:128], wup_sb[:, 128:640],
                                     start=True, stop=True)

            for t_sb, t_in in ((w4_sb, w4_in), (ident_sb, ident_in),
                               (xd_sb, xd_in), (wxp_sb, wxp_in)):
                nc.sync.dma_start(t_sb[:], t_in[:])
            dts = []
            with tc.tile_pool(name="dpool", bufs=1) as dpool_pre:
                for c in range(16):
                    dtile = dpool_pre.tile([128, H], f16, tag=f"d{c}", name=f"d{c}")
                    nc.sync.dma_start(dtile[:], d_in[c])
                    dts.append(dtile)
            for t_sb, t_in in ((w2tw_sb, w2tw_in), (w2ta_sb, w2ta_in),
                               (b1_sb, b1_in), (b2a_sb, b2a_in), (b2w_sb, b2w_in),
                               (ta_sb, ta_in), (selu_sb, selu_in),
                               (sel2_sb, sel2_in),
                               (m1fa_sb, m1fa_in), (m1fb_sb, m1fb_in),
                               (icj_sb, icj_in),
                               (tailc_sb, tailc_in), (sel48_sb, sel48_in),
                               (efix_sb, efix_in), (bias_sb, bias_in)):
                nc.sync.dma_start(t_sb[:], t_in[:])

            # persistent SBUF tiles
            h_sb = cpool.tile([32, F], f16, tag="h")
            sA_sb = cpool.tile([128, H], f16, tag="sA")
            mv_sb = cpool.tile([33, H], f16, tag="mv")        # wid rows + ones
            mv2_sb = cpool.tile([64, H], f16, tag="mv2")      # wioa + ra rows
            amp0_sb = cpool.tile([64, F], f32, tag="amp0")
            ra32_sb = cpool.tile([64, F], f32, tag="ra32")
            ra16_sb = cpool.tile([32, F], f16, tag="ra16")
            xa_sb = cpool.tile([128, 2 * F], f16, tag="xa")
            xb_sb = cpool.tile([128, 2 * F], f16, tag="xb")
            rta_sb = cpool.tile([128, 8], f16, tag="rta")     # R tail (a), col=oi
            rtb_sb = cpool.tile([128, 8], f16, tag="rtb")
            utp_sb = cpool.tile([32, 128], f16, tag="utp")
            tcol_sb = cpool.tile([32, 4], f32, tag="tcol")
            snq_sb = cpool.tile([32, 128], f16, tag="snq")
            u2a_sb = cpool.tile([32, 128], f32, tag="u2a")
            u2b_sb = cpool.tile([32, 128], f32, tag="u2b")
            ftq_sb = cpool.tile([32, 128], bf16, tag="ftq")
            ft8_sb = cpool.tile([16, 128], f16, tag="ft8")
            fttT_sb = cpool.tile([128, 16], f16, tag="fttT")
            yt_sb = cpool.tile([64, 4096], f32, tag="yt")
            ya_t = [cpool.tile([128, H], f16, tag=f"ya{o}", name=f"ya{o}") for o in range(4)]
            yb_t = [cpool.tile([128, H], f16, tag=f"yb{o}", name=f"yb{o}") for o in range(4)]
            hp1 = cpool.tile([32, F], f16, tag="hp1")
            hp2 = cpool.tile([32, F], f16, tag="hp2")

            # ---- stage 1a: conv with j packed in PE output ----
            with tc.tile_pool(name="ps1a", bufs=2, space="PSUM") as ps1a:
                for (f0, nf) in [(0, 512), (512, 512), (1024, 4)]:
                    ps = ps1a.tile([128, 512], f32, tag="ps1a")
                    for cb in range(16):
                        nc.tensor.matmul(
                            ps[:, 0:nf],
                            w4_sb[:, cb * 128:(cb + 1) * 128],
                            dts[cb][:, f0:f0 + nf],
                            start=(cb == 0), stop=(cb == 15))
                    nc.scalar.activation(sA_sb[:, f0:f0 + nf], ps[:, 0:nf],
                                         AF.Copy)

            # ---- stage 1b: fold 4 shifted j groups via PE; Lrelu(x + b1) ----
            # chunk edges chosen so fold chunk k only reads sA chunks <= k
            # (each fold chunk's copy overlaps the next chunk's matmuls)
            with tc.tile_pool(name="psf", bufs=2, space="PSUM") as psf:
                for (f0, nf) in [(0, 509), (509, 512), (1021, 4)]:
                    ph = psf.tile([32, 512], f32, tag="psf")
                    for j in range(4):
                        nc.tensor.matmul(ph[:, 0:nf],
                                         ident_sb[:, 32 * j:32 * (j + 1)],
                                         sA_sb[:, f0 + j:f0 + j + nf],
                                         start=(j == 0), stop=(j == 3))
                    nc.vector.tensor_scalar(hp1[:, f0:f0 + nf], ph[:, 0:nf],
                                            b1_sb[:, 0:1], None,
                                            mybir.AluOpType.add)
            nc.vector.tensor_scalar(hp2[:], hp1[:], 0.01, None,
                                    mybir.AluOpType.mult)
            nc.vector.tensor_max(h_sb[:], hp1[:], hp2[:])

            # zero the pad columns of ya/yb (cmul writes 1..1025 only)
            nc.gpsimd.memset(mv_sb[32:33, :], 1.0)
            for o in range(4):
                for yd in (ya_t[o], yb_t[o]):
                    nc.gpsimd.memset(yd[:, 0:1], 0.0)
                    nc.gpsimd.memset(yd[:, 1026:1028], 0.0)

            # ---- STFT halves around stage2 + tail chain: the PE streams
            # STFT matmuls while scalar/vector run tanh -> recip -> mv2 ----
            def stft_half(ps4, i, cp_i):
                for (mt, xdst) in ((0, xa_sb), (1, xb_sb)):
                    for (f0, nf) in FCH:
                        ps = ps4.tile([128, 512], f32, tag="ps4")
                        for j in range(4):
                            nc.tensor.matmul(
                                ps[:, 0:nf],
                                wxp_sb[:, (mt * 4 + j) * 128:(mt * 4 + j + 1) * 128],
                                xd_sb[:, i * H + f0 + j:i * H + f0 + j + nf],
                                start=(j == 0), stop=(j == 3))
                        if cp_i % 2 == 0:
                            nc.vector.tensor_copy(
                                xdst[:, i * F + f0:i * F + f0 + nf], ps[:, 0:nf])
                        else:
                            nc.scalar.activation(
                                xdst[:, i * F + f0:i * F + f0 + nf],
                                ps[:, 0:nf], AF.Copy)
                        cp_i += 1
                return cp_i

            with tc.tile_pool(name="ps4", bufs=2, space="PSUM") as ps4, \
                 tc.tile_pool(name="ps2", bufs=2, space="PSUM") as ps2, \
                 tc.tile_pool(name="pstl", bufs=1, space="PSUM") as pstl:
                cp_i = stft_half(ps4, 0, 0)

                # stage 2: 1x1 conv + tanh -> wid (mv rows 0-31), amp0
                for (f0, nf) in [(0, 512), (512, 512), (1024, 1)]:
                    pw = ps2.tile([32, 512], f32, tag="ps2w")
                    nc.tensor.matmul(pw[:, 0:nf], w2tw_sb[:], h_sb[:, f0:f0 + nf],
                                     start=True, stop=True)
                    nc.scalar.activation(mv_sb[0:32, f0:f0 + nf], pw[:, 0:nf],
                                         AF.Tanh, bias=b2w_sb[:, 0:1])
                    pa = ps2.tile([64, 512], f32, tag="ps2a")
                    nc.tensor.matmul(pa[:, 0:nf], w2ta_sb[:], h_sb[:, f0:f0 + nf],
                                     start=True, stop=True)
                    nc.scalar.activation(amp0_sb[:, f0:f0 + nf], pa[:, 0:nf],
                                         AF.Tanh, bias=b2a_sb[:, 0:1])

                # mv prep: ra = clamp(1/amp), wioa = wid*ra (overlaps STFT i=1)
                nc.vector.reciprocal_approx_fast(ra32_sb[:], amp0_sb[:])
                nc.vector.tensor_scalar_min(ra32_sb[:], ra32_sb[:], CLAMP)
                nc.vector.tensor_scalar_max(ra16_sb[:], ra32_sb[0:32, :], -CLAMP)
                nc.vector.tensor_scalar_max(mv2_sb[32:64, 0:F],
                                            ra32_sb[32:64, :], -CLAMP)
                nc.vector.tensor_mul(mv2_sb[0:32, 0:F], mv_sb[0:32, 0:F],
                                     ra16_sb[:])

                cp_i = stft_half(ps4, 1, cp_i)
                # STFT tail f=1024 (batched: cols = i*2 + mt)
                pst = ps4.tile([128, 4], f32, tag="ps4")
                for i in range(2):
                    for mt in range(2):
                        for j in range(4):
                            nc.tensor.matmul(
                                pst[:, i * 2 + mt:i * 2 + mt + 1],
                                wxp_sb[:, (mt * 4 + j) * 128:(mt * 4 + j + 1) * 128],
                                xd_sb[:, i * H + 1024 + j:i * H + 1024 + j + 1],
                                start=(j == 0), stop=(j == 3))
                for i in range(2):
                    nc.scalar.activation(xa_sb[:, i * F + 1024:i * F + 1025],
                                         pst[:, i * 2:i * 2 + 1], AF.Copy)
                    nc.scalar.activation(xb_sb[:, i * F + 1024:i * F + 1025],
                                         pst[:, i * 2 + 1:i * 2 + 2], AF.Copy)

                # f=1024 tail on the DVE: u' = ta*wid + PIEPS (per q row),
                # u2 = S2*(wioa + ra*epsova); ft = sin(u')*approx(1/u2);
                # one selector matmul sums s and a transpose-DMA flips to
                # [d, oi] for the tail DFT.  No tiny-matmul storm.
                nc.vector.memset(tcol_sb[:, 3:4], PIEPS)
                nc.vector.tensor_copy(tcol_sb[:, 0:1], mv_sb[0:32, 1024:1025])
                nc.vector.tensor_copy(tcol_sb[:, 1:2], ra16_sb[:, 1024:1025])
                nc.vector.tensor_copy(tcol_sb[:, 2:3], mv2_sb[0:32, 1024:1025])
                nc.vector.tensor_scalar(utp_sb[:], tailc_sb[:, 0:128],
                                        tcol_sb[:, 0:1], None,
                                        mybir.AluOpType.mult)
                nc.scalar.activation(snq_sb[:], utp_sb[:], AF.Sin,
                                     bias=tcol_sb[:, 3:4])
                nc.vector.tensor_scalar(u2a_sb[:], tailc_sb[:, 256:384],
                                        tcol_sb[:, 1:2], None,
                                        mybir.AluOpType.mult)
                nc.vector.scalar_tensor_tensor(
                    u2b_sb[:], tailc_sb[:, 128:256],
                    tcol_sb[:, 2:3], u2a_sb[:],
                    mybir.AluOpType.mult, mybir.AluOpType.add)
                nc.vector._custom_dve(divop, out=ftq_sb[:], in0=u2b_sb[:],
                                      in1=snq_sb[:], s0=DIV_C0, s1=DIV_C1,
                                      imm2=0.0)
                pqt = pstl.tile([16, 128], f32, tag="pqt")
                nc.tensor.matmul(pqt[:], sel48_sb[:], ftq_sb[:],
                                 start=True, stop=True)
                nc.scalar.activation(ft8_sb[:], pqt[:], AF.Copy)
                nc.sync.dma_start_transpose(fttT_sb[:], ft8_sb[:])

            # ---- main loop: u/u2 -> sin/recip -> ft -> presum -> DFT -> cmul ----
            with tc.tile_pool(name="psu", bufs=2, space="PSUM") as psu, \
                 tc.tile_pool(name="psv", bufs=3, space="PSUM") as psv, \
                 tc.tile_pool(name="psr", bufs=2, space="PSUM") as psr, \
                 tc.tile_pool(name="psy", bufs=1, space="PSUM") as psy, \
                 tc.tile_pool(name="snp", bufs=5) as snp, \
                 tc.tile_pool(name="ftp", bufs=10) as ftp, \
                 tc.tile_pool(name="ctp", bufs=3) as ctp:
                def emit_chain(oi):
                    fts = []
                    for s in range(4):
                        q = oi * 4 + s
                        pvs = []
                        for (f0, nf) in FCH:
                            pv = psv.tile([128, 512], f32, tag="pv")
                            nc.tensor.matmul(pv[:, 0:nf],
                                             sel2_sb[:, q * 128:(q + 1) * 128],
                                             mv2_sb[0:64, f0:f0 + nf],
                                             start=True, stop=True)
                            pvs.append(pv)
                        sn = snp.tile([128, 1024], f16, tag="sn")
                        for (f0, nf) in FCH:
                            pu = psu.tile([128, 512], f32, tag="pu")
                            nc.tensor.matmul(pu[:, 0:nf],
                                             selu_sb[:, q * 128:(q + 1) * 128],
                                             mv_sb[0:33, f0:f0 + nf],
                                             start=True, stop=True)
                            nc.scalar.activation(sn[:, f0:f0 + nf], pu[:, 0:nf],
                                                 AF.Sin, scale=ta_sb[:, 0:1])
                        ft = ftp.tile([128, 1024], bf16, tag="ft")
                        for ci, (f0, nf) in enumerate(FCH):
                            nc.vector._custom_dve(divop, out=ft[:, f0:f0 + nf],
                                                  in0=pvs[ci][:, 0:nf],
                                                  in1=sn[:, f0:f0 + nf],
                                                  s0=DIV_C0, s1=DIV_C1, imm2=0.0)
                        fts.append(ft)
                    return fts

                def emit_dft_cmul(oi, fts):
                    o, il = oi // 2, oi % 2
                    for (mt, m1sb, rt, xsb, ydst) in (
                            (0, m1fa_sb, rta_sb, xa_sb, ya_t[o]),
                            (1, m1fb_sb, rtb_sb, xb_sb, yb_t[o])):
                        for (f0, nf) in FCH:
                            ps = psr.tile([128, 512], f32, tag="psr")
                            for s in range(4):
                                nc.tensor.matmul(ps[:, 0:nf], m1sb[:],
                                                 fts[s][:, f0:f0 + nf],
                                                 start=(s == 0), stop=(s == 3))
                            # cmul straight from PSUM (R is real)
                            if il == 0:
                                nc.vector.tensor_mul(
                                    ydst[:, 1 + f0:1 + f0 + nf],
                                    xsb[:, il * F + f0:il * F + f0 + nf],
                                    ps[:, 0:nf])
                            else:
                                tmp = ctp.tile([128, 512], f16, tag="ctmp")
                                nc.vector.tensor_mul(
                                    tmp[:, 0:nf],
                                    xsb[:, il * F + f0:il * F + f0 + nf],
                                    ps[:, 0:nf])
                                nc.gpsimd.tensor_add(ydst[:, 1 + f0:1 + f0 + nf],
                                                     ydst[:, 1 + f0:1 + f0 + nf],
                                                     tmp[:, 0:nf])
                        # f = 1024 tail column from rta/rtb
                        if il == 0:
                            nc.vector.tensor_mul(ydst[:, 1025:1026],
                                                 xsb[:, il * F + 1024:il * F + 1025],
                                                 rt[:, oi:oi + 1])
                        else:
                            tmp = ctp.tile([128, 512], f16, tag="ctmp")
                            nc.vector.tensor_mul(tmp[:, 0:1],
                                                 xsb[:, il * F + 1024:il * F + 1025],
                                                 rt[:, oi:oi + 1])
                            nc.gpsimd.tensor_add(ydst[:, 1025:1026],
                                                 ydst[:, 1025:1026], tmp[:, 0:1])

                def emit_istft(o):
                    for (c0, nf) in FCH:
                        py = psy.tile([64, 512], f32, tag="py")
                        for jj in range(4):
                            for (ab, ysrc) in ((0, ya_t), (1, yb_t)):
                                st = icj_sb[:, (ab * 4 + jj) * 64:(ab * 4 + jj + 1) * 64]
                                nc.tensor.matmul(
                                    py[:, 0:nf], st,
                                    ysrc[o][:, c0 + 3 - jj:c0 + 3 - jj + nf],
                                    start=(jj == 0 and ab == 0),
                                    stop=(jj == 3 and ab == 1))
                        if c0 == 0:
                            nc.vector.tensor_scalar(py[:, 0:1], py[:, 0:1],
                                                    efix_sb[:, 0:1], None,
                                                    mybir.AluOpType.mult)
                        else:
                            nc.vector.tensor_scalar(py[:, 511:512], py[:, 511:512],
                                                    efix_sb[:, 1:2], None,
                                                    mybir.AluOpType.mult)
                        nc.scalar.activation(
                            yt_sb[:, o * 1024 + c0:o * 1024 + c0 + nf],
                            py[:, 0:nf],
                            AF.Identity, bias=bias_sb[:, o:o + 1])
                        nc.sync.dma_start(
                            yt_out[:, o * 1024 + c0:o * 1024 + c0 + nf],
                            yt_sb[:, o * 1024 + c0:o * 1024 + c0 + nf])

                prev_fts = emit_chain(0)
                # tail DFT: rta/rtb from the s-summed ftt4 (PE already busy
                # with chain(0) while scalar/DVE produced ftt4)
                prt = psr.tile([128, 512], f32, tag="psr")
                nc.tensor.matmul(prt[:, 0:8], m1fa_sb[:], fttT_sb[:, 0:8],
                                 start=True, stop=True)
                nc.tensor.matmul(prt[:, 8:16], m1fb_sb[:], fttT_sb[:, 0:8],
                                 start=True, stop=True)
                nc.scalar.activation(rta_sb[:], prt[:, 0:8], AF.Copy)
                nc.scalar.activation(rtb_sb[:], prt[:, 8:16], AF.Copy)

                for oi in range(1, 8):
                    fts = emit_chain(oi)
                    emit_dft_cmul(oi - 1, prev_fts)
                    if (oi - 1) in (2, 4, 6):
                        emit_istft((oi - 1) // 2 - 1)
                    prev_fts = fts
                emit_dft_cmul(7, prev_fts)
                emit_istft(3)

    nc.compile()
    return nc


def _prep_inputs(x, conditioning, w1, b1, w2, b2, bias):
    c = _consts()
    x = np.asarray(x, dtype=np.float32)
    conditioning = np.asarray(conditioning, dtype=np.float32)
    w1 = np.asarray(w1, dtype=np.float32)
    b1 = np.asarray(b1, dtype=np.float32)
    w2 = np.asarray(w2, dtype=np.float32)
    b2 = np.asarray(b2, dtype=np.float32)
    bias = np.asarray(bias, dtype=np.float32)

    # W4[cb, 64*(c-2cb)+r, 32j+o] = w1[o, c, 64j+r]
    arr = w1.reshape(32, 32, 4, 64)                      # [o, c, j, r]
    W4 = arr.transpose(1, 3, 2, 0).reshape(32, 64, 4 * 32)  # [c, r, (j,o)]
    W4 = W4.reshape(16, 128, 128)                        # [cb, row, col]
    w4 = np.ascontiguousarray(
        W4.transpose(1, 0, 2).reshape(128, 2048)).astype(np.float16)

    w2m = w2[:, :, 0]                                    # [64, 32]
    w2tw = np.ascontiguousarray(w2m[32:64].T).astype(np.float16)
    w2ta1 = w2m[0:32].T                                  # [32, 32]
    w2ta = np.ascontiguousarray(
        np.concatenate([w2ta1, w2ta1], axis=1)).astype(np.float16)  # [32, 64]
    bias64 = np.tile(bias.reshape(1, 4), (64, 1)).astype(np.float32)

    shared = {
        "w4_in": w4,
        "w2tw_in": w2tw,
        "w2ta_in": w2ta,
        "ident_in": c["ident"],
        "b1_in": b1.reshape(32, 1).copy(),
        "b2a_in": np.tile(b2[:32], 2).reshape(64, 1).copy(),
        "b2w_in": b2[32:].reshape(32, 1).copy(),
        "ta_in": c["scale_ta"],
        "selu_in": c["selu"], "sel2_in": c["sel2"],
        "m1fa_in": c["m1fa"], "m1fb_in": c["m1fb"],
        "wxp_in": c["wxp"], "icj_in": c["icj"], "efix_in": c["efix"],
        "tailc_in": c["tailc"], "sel48_in": c["sel48"],
        "bias_in": bias64,
    }
    in_maps = []
    for b in range(B):
        condpad = np.zeros((CI, T + K), dtype=np.float32)
        condpad[:, 128:128 + T] = conditioning[b]
        d = condpad.reshape(CI, H, 64).transpose(0, 2, 1).reshape(2048, H)
        d = np.ascontiguousarray(d.reshape(16, 128, H)).astype(np.float16)
        xp = np.pad(x[b], ((0, 0), (128, 128)), mode="reflect")
        xd = np.ascontiguousarray(
            xp.reshape(2, H, 64).transpose(0, 2, 1).reshape(2, 64, H)
            .transpose(1, 0, 2).reshape(64, 2 * H)).astype(np.float16)
        m = dict(shared)
        m["d_in"] = d
        m["xd_in"] = xd
        in_maps.append(m)
    return in_maps


def _assemble(results):
    y = np.empty((B, O, T), dtype=np.float32)
    for b in range(B):
        yt = results[b]["yt_out"]                        # [64, 4096]
        y[b] = yt.reshape(64, 4, 1024).transpose(1, 2, 0).reshape(4, T)
    return y


def kernel(x, conditioning, w1, b1, w2, b2, bias):
    from concourse.bass_utils import run_bass_kernel_spmd
    if "nc" not in _prog_cache:
        _prog_cache["nc"] = _build_program()
    nc = _prog_cache["nc"]
    in_maps = _prep_inputs(x, conditioning, w1, b1, w2, b2, bias)
    res = run_bass_kernel_spmd(nc, in_maps, core_ids=list(range(B)))
    return _assemble(res.results)
